# revision 23
# baseline (speedup 1.0000x reference)
"""Trainium2 Bass kernel for nn_MemorizingTransformer (retrieval_knn).

Memorizing-transformer attention block: cosine-sim causal local attention with
per-query retrieved KNN memories, joint softmax over [memory | local], and
input/output projections.

Sharding: (b, h) across 8 cores — core c handles batch b=c//4 and heads
h0=2*(c%4), h0+1. Every core runs an identical NEFF (pure SPMD); only input
slices differ. The output projection is computed per-core on the core's two
head rows of w_out, giving fp16 partial sums that the host reduces.

Precision plan (validated against the reference on CPU, rel-err ~5e-3 vs the
2e-2 gate): fp16 for the score path (x, w_q/w_kv, q̂, k, mem_k, mem-score
products and their f16-accumulated reduce), bf16 for everything downstream of
exp (probabilities, v, mem_v, mem-value products), f32 for PSUM accumulation
and softmax denominators. exp outputs must be bf16, not fp16: scores span
exp(-2*scale) .. 1 which underflows fp16's exponent range.

Device algorithm per core (all 16-bit matmuls, 1 cycle/row on PE):
  proj   : qkv = xT-chunks @ wqkv (PE, f32 psum); ACT copies to SBUF f16/bf16;
           sumsq via DVE tensor_tensor_reduce; q normalized on DVE.
           k stays unnormalized — 1/|k_j| is folded into the exp scale, which
           is per-partition (per j) in the S_T = [j, q] layout.
  qT/kT  : PE transposes (fp16 identity), psum -> SBUF via gpsimd copies.
  per head p, per query-quad qc (4 blocks of 128 queries):
    memk : DMA mk f16 -> DVE mul with q̂ (2x) -> DVE f16 reduce (2x) -> ACT
           exp -> P_mem bf16.
    local: per jt <= 4qc+3: S_T[128j, <=512q] = kT_jt.T @ qT (PE), ACT exp
           with scale=sc/|k_j| bias=-sc -> p_t bf16, tril mask on the
           diagonal block (DVE), PV: psum_o[g] += p_t.T @ [v|1] (PE accum,
           stop at jt==g).
    memv : DMA mv bf16 (d-major) -> DVE mul with P_mem -> DVE reduce -> mem
           numerator; denominator joined via tensor_tensor_reduce seeded with
           the local denominator from psum col 64; oh = (psum+mo)*rcp (DVE).
    ohT  : PE transposes -> hoT f16.
  outproj: partial[g] = hoT_g.T @ w_out rows (PE), ACT copy f16, DMA out.

Softmax needs no max-subtraction: scores are cosine sims in [-1,1] times
scale=exp(scale_param), so exp(scale*(s-1)) is bounded in (0, 1].
"""

import os
import numpy as np

HEADS = 8
D = 64
KNN = 32
B = 2
N = 2048
DIM = 512
P = 128
NB = N // P          # 16 query/key blocks
NCO = DIM // P       # 4 contraction chunks of the model dim
NCORES = 8
REDV_F32 = bool(int(os.environ.get("BASS_REDV_F32", "0")))
MK_G = int(os.environ.get("BASS_MK_G", "14"))   # memk reduces on gpsimd (of 32)
MV_G = int(os.environ.get("BASS_MV_G", "14"))   # memv reduces on gpsimd (of 32)
PHASE_MARKS = []
_MSTATE = {}


def _mark(nc, name):
    cur = nc.next_id()
    if _MSTATE.get("name") is not None:
        PHASE_MARKS.append((_MSTATE["name"], _MSTATE["id"], cur))
    _MSTATE["name"] = name
    _MSTATE["id"] = cur


def _build(use_mbias: bool):
    import concourse.bass as bass
    import concourse.mybir as mybir
    import concourse.tile as tile
    from concourse import bacc

    f32 = mybir.dt.float32
    f16 = mybir.dt.float16
    bf16 = mybir.dt.bfloat16
    mvdt = f32 if REDV_F32 else bf16
    AX = mybir.AxisListType
    ACTF = mybir.ActivationFunctionType
    ALU = mybir.AluOpType

    nc = bacc.Bacc(None, target_bir_lowering=False, name="memxformer")
    PHASE_MARKS.clear()
    _MSTATE.clear()

    # ---- I/O ------------------------------------------------------------
    xt = nc.dram_tensor("xt", (P, NCO, N), f16, kind="ExternalInput")
    wqkv = nc.dram_tensor("wqkv", (P, NCO, 4 * D), f16, kind="ExternalInput")
    wout2 = nc.dram_tensor("wout2", (2 * D, DIM), f16, kind="ExternalInput")
    # scvec cols: [sc0, sc1, -sc0, -sc1, 1/sc0^2, 1/sc1^2, 0, 0]
    scvec = nc.dram_tensor("scvec", (P, 8), f32, kind="ExternalInput")
    mk = nc.dram_tensor("mk", (2, NB, P, KNN, D), f16, kind="ExternalInput")
    mv = nc.dram_tensor("mv", (2, NB, P, KNN, D + 1), bf16, kind="ExternalInput")
    if use_mbias:
        mbias = nc.dram_tensor("mbias", (2, NB, P, KNN), f32, kind="ExternalInput")
    out = nc.dram_tensor("out", (N, DIM), f16, kind="ExternalOutput")

    # constants baked into the NEFF
    import ml_dtypes
    eye16_d = nc.inline_tensor(np.eye(P, dtype=np.float16), name="eye16_c")
    eyebf_d = nc.inline_tensor(np.eye(P, dtype=ml_dtypes.bfloat16), name="eyebf_c")
    eye32_d = nc.inline_tensor(np.eye(D + 1, dtype=np.float32), name="eye32_c")
    # keep j <= q in the S_T = [j, q] layout
    triu_d = nc.inline_tensor(
        np.triu(np.ones((P, P), dtype=np.float32)).astype(ml_dtypes.bfloat16),
        name="triu_c")


    with tile.TileContext(nc) as tc:
        with (
            tc.tile_pool(name="singles", bufs=1) as singles,
            tc.tile_pool(name="mkp", bufs=2) as mkp,
            tc.tile_pool(name="mvp", bufs=2) as mvp,
            tc.tile_pool(name="prodp", bufs=2) as prodp,
            tc.tile_pool(name="treep", bufs=1) as treep,
            tc.tile_pool(name="ptp", bufs=3) as ptp,
            tc.tile_pool(name="stts", bufs=2) as stts,
            tc.tile_pool(name="pms", bufs=2) as pms,
            tc.tile_pool(name="smallp", bufs=6) as smallp,
            tc.tile_pool(name="outp", bufs=2) as outp,
            tc.tile_pool(name="ppt", bufs=2, space="PSUM") as ppt,
            tc.tile_pool(name="pp512", bufs=2, space="PSUM") as pp512,
            tc.tile_pool(name="ppo", bufs=4, space="PSUM") as ppo,
        ):
            # ---- constants / weights ------------------------------------
            eye_sb = singles.tile([P, P], f16, tag="eye16")
            nc.sync.dma_start(eye_sb, eye16_d[:, :])
            eyebf_sb = singles.tile([P, P], bf16, tag="eyebf")
            nc.sync.dma_start(eyebf_sb, eyebf_d[:, :])
            eye32_sb = singles.tile([D + 1, D + 1], f32, tag="eye32")
            nc.sync.dma_start(eye32_sb, eye32_d[:, :])
            tril_sb = singles.tile([P, P], bf16, tag="tril")
            nc.sync.dma_start(tril_sb, triu_d[:, :])
            sc_sb = singles.tile([P, 8], f32, tag="scales")
            nc.sync.dma_start(sc_sb, scvec[:, :])
            wqkv_sb = singles.tile([P, NCO, 4 * D], f16, tag="wqkv")
            nc.sync.dma_start(wqkv_sb, wqkv[:, :, :])
            wout_sb = singles.tile([P, DIM], f16, tag="wout")
            nc.sync.dma_start(wout_sb, wout2[:, :])
            xT = singles.tile([P, NCO, N], f16, tag="xT")
            nc.sync.dma_start(xT, xt[:, :, :])

            _mark(nc, "setup")
            # ---- PE warm-up: ~5us of dense matmul raises the HAM clock
            # gate (K=4/8 -> 8/8, 1.2 -> 2.4 GHz) before the real work
            wm_ps = pp512.tile([P, 512], f32, tag="st", name="wm_ps")
            for wi in range(12):
                nc.tensor.matmul(wm_ps, wqkv_sb[:, wi % NCO, 0:P],
                                 xT[:, wi % NCO, 0:512],
                                 start=True, stop=True)
            wjunk = singles.tile([P, 512], f16, tag="wjunk")
            nc.scalar.copy(out=wjunk, in_=wm_ps)

            # ---- projections: qkv = x @ [wq2 | wk | wv] -----------------
            # qkv16 cols: [q0 64 | q1 64 | k 64]; v goes straight to v_aug
            qkv16 = singles.tile([P, NB, 3 * D], f16, tag="qkv16")
            v_aug = singles.tile([P, NB, D + 1], bf16, tag="vaug")
            nc.gpsimd.memset(v_aug[:, :, D:D + 1], 1.0)
            # ss cols: 0:32 q-blocks (2 heads x 16), 32:48 k-blocks
            ss = singles.tile([P, 3 * NB], f32, tag="ss")
            junk = singles.tile([P, D], f16, tag="junk")

            for g in range(NB):
                qsl = slice(g * P, (g + 1) * P)
                qkv_ps = ppt.tile([P, 4 * D], f32, tag="tps")
                for co in range(NCO):
                    nc.tensor.matmul(qkv_ps, xT[:, co, qsl], wqkv_sb[:, co, :],
                                     start=(co == 0), stop=(co == NCO - 1))
                nc.scalar.copy(out=qkv16[:, g, :], in_=qkv_ps[:, 0:3 * D])
                nc.scalar.copy(out=v_aug[:, g, 0:D], in_=qkv_ps[:, 3 * D:4 * D])
                for s3 in range(3):
                    col = s3 * NB + g if s3 < 2 else 2 * NB + g
                    seg = qkv16[:, g, s3 * D:(s3 + 1) * D]
                    nc.scalar.activation(out=junk, in_=seg, func=ACTF.Square,
                                         accum_out=ss[:, col:col + 1])

            _mark(nc, "proj")
            # ---- norms --------------------------------------------------
            # q: q_s = q / |q|;  k: fold sc/|k_j| into the local exp scale
            nrm_q = singles.tile([P, 2 * NB], f32, tag="nrm_q")
            nc.scalar.sqrt(nrm_q, ss[:, 0:2 * NB])
            rn_q = singles.tile([P, 2 * NB], f32, tag="rn_q")
            nc.vector.reciprocal(rn_q, nrm_q)
            scexp = singles.tile([P, 2, NB], f32, tag="scexp")
            sktmp = singles.tile([P, 2, NB], f32, tag="sktmp")
            for p in range(2):
                nc.scalar.activation(out=sktmp[:, p, :],
                                     in_=ss[:, 2 * NB:3 * NB],
                                     func=ACTF.Sqrt,
                                     scale=sc_sb[:, 4 + p:5 + p])
                nc.vector.reciprocal(scexp[:, p, :], sktmp[:, p, :])

            q_s = singles.tile([P, 2 * NB, D], f16, tag="q_s")
            for idx in range(2 * NB):
                p, g = idx // NB, idx % NB
                nc.vector.tensor_scalar_mul(q_s[:, idx, :],
                                            qkv16[:, g, p * D:(p + 1) * D],
                                            rn_q[:, idx:idx + 1])

            _mark(nc, "norm")
            # ---- transposes: qT, kT -------------------------------------
            qT = singles.tile([D, 2 * NB, P], f16, tag="qT")
            for idxc in range(0, 2 * NB, 4):
                qt_ps = ppt.tile([D, 4, P], f16, tag="tps")
                for i4 in range(4):
                    nc.tensor.transpose(qt_ps[:, i4, :], q_s[:, idxc + i4, :],
                                        eye_sb)
                nc.scalar.copy(out=qT[:, idxc:idxc + 4, :], in_=qt_ps)
            kT = singles.tile([D, NB, P], f16, tag="kT")
            for jtc in range(0, NB, 4):
                kt_ps = ppt.tile([D, 4, P], f16, tag="tps")
                for j4 in range(4):
                    nc.tensor.transpose(kt_ps[:, j4, :],
                                        qkv16[:, jtc + j4, 2 * D:3 * D], eye_sb)
                nc.scalar.copy(out=kT[:, jtc:jtc + 4, :], in_=kt_ps)

            _mark(nc, "qkT")
            # ---- attention ----------------------------------------------
            hoT = singles.tile([P, NB, P], f16, tag="hoT")
            pm_all = singles.tile([P, 2, NB, KNN], bf16, tag="pm_all")
            # block-diagonal staging for P_mem (the off-diagonal zeros are
            # written once and never touched again; double-buffered per quad)
            st2 = singles.tile([P, 2, 4, P], bf16, tag="st2")
            nc.gpsimd.memset(st2, 0.0)

            for p in range(2):
                sc_ap = sc_sb[:, p:p + 1]
                nb_ap = sc_sb[:, 2 + p:3 + p]

                def memk_quad(qc):
                    gc = 4 * qc
                    mk_t = mkp.tile([P, 4, KNN, D], f16, tag="mk", name="mk_t")
                    nc.sync.dma_start(
                        mk_t, mk[p, gc:gc + 4].rearrange("g p j d -> p g j d"))
                    prod = prodp.tile([P, 4, KNN, D], f16, tag="prod",
                                      name="prod")
                    nc.vector.tensor_mul(
                        prod, mk_t,
                        q_s[:, p * NB + gc:p * NB + gc + 4, None, :]
                        .to_broadcast((P, 4, KNN, D)))
                    t1 = treep.tile([P, 4, KNN, D // 2], f16, tag="t1",
                                    name="t1")
                    nc.vector.tensor_add(t1, prod[:, :, :, 0:D // 2],
                                         prod[:, :, :, D // 2:D])
                    t2 = treep.tile([P, 4, KNN, D // 4], f16, tag="t2",
                                    name="t2")
                    nc.vector.tensor_add(t2, t1[:, :, :, 0:D // 4],
                                         t1[:, :, :, D // 4:D // 2])
                    s_mem = smallp.tile([P, 4, KNN], f32, tag="smem",
                                        name="s_mem")
                    nc.vector.reduce_sum(s_mem, t2, axis=AX.X)
                    if use_mbias:
                        mb_t = smallp.tile([P, 4, KNN], f32, tag="mbias",
                                           name="mb_t")
                        nc.sync.dma_start(
                            mb_t,
                            mbias[p, gc:gc + 4].rearrange("g p j -> p g j"))
                        nc.vector.tensor_add(s_mem, s_mem, mb_t)
                    nc.scalar.activation(out=pm_all[:, p, gc:gc + 4, :],
                                         in_=s_mem, func=ACTF.Exp,
                                         bias=nb_ap, scale=sc_ap)

                def local_tile(qc, jt, psum_o):
                    g_lo = max(jt, 4 * qc)
                    g_hi = 4 * qc + 4
                    ng = g_hi - g_lo
                    i_lo = p * NB + g_lo
                    st_ps = pp512.tile([P, 512], f32, tag="st", name="st_ps")
                    nc.tensor.matmul(
                        st_ps[:, :ng * P], kT[:, jt, :],
                        qT[:, i_lo:i_lo + ng, :], start=True, stop=True)
                    p_t = ptp.tile([P, 4, P], bf16, tag="pt", name="p_t")
                    nc.scalar.activation(
                        out=p_t[:, :ng, :],
                        in_=st_ps[:, :ng * P].rearrange("p (g q) -> p g q", q=P),
                        func=ACTF.Exp, bias=nb_ap, scale=scexp[:, p, jt:jt + 1])
                    if g_lo <= jt < g_hi:
                        di = jt - g_lo
                        nc.vector.tensor_mul(p_t[:, di, :], p_t[:, di, :],
                                             tril_sb)
                    for gi in range(ng):
                        g = g_lo + gi
                        nc.tensor.matmul(
                            psum_o[:, g - 4 * qc, :], p_t[:, gi, :],
                            v_aug[:, jt, :],
                            start=(jt == 0 and gi == 0), stop=False)

                _mark(nc, "attn")
                for qc in range(4):
                    memk_quad(qc)
                    gc = 4 * qc
                    # mv prefetch + P_mem staging run on DMA/gpsimd/PE ahead
                    # of (and overlapped with) the local S/PV chain
                    mv_t = mvp.tile([P, 4, KNN, D + 1], bf16, tag="mv",
                                    name="mv_t")
                    nc.sync.dma_start(
                        mv_t, mv[p, gc:gc + 4].rearrange("g p q d -> p g q d"))
                    stage4 = st2[:, qc % 2, :, :]
                    for gi in range(4):
                        g = gc + gi
                        for k4 in range(4):
                            nc.gpsimd.tensor_copy(
                                out=stage4[32 * k4:32 * (k4 + 1), gi,
                                           32 * k4:32 * (k4 + 1)],
                                in_=pm_all[32 * k4:32 * (k4 + 1), p, g, :])
                    stt_ps = ppt.tile([P, 4, P], bf16, tag="tps", name="stt_ps")
                    for gi in range(4):
                        nc.tensor.transpose(stt_ps[:, gi, :], stage4[:, gi, :],
                                            eyebf_sb)
                    stT = stts.tile([P, 4, P], bf16, tag="stT", name="stT")
                    nc.scalar.copy(out=stT, in_=stt_ps)
                    psum_o = ppo.tile([P, 4, D + 1], f32, tag="po",
                                      name=f"po{qc}")
                    for jt in range(4 * qc + 4):
                        local_tile(qc, jt, psum_o)
                    # pm_ps columns are (g4, ql) so each tiny matmul writes
                    # 4 contiguous columns; the accumulate-transpose below
                    # restores query order ql*32+g4 via a rearranged AP
                    pm_ps = pp512.tile([D + 1, 4, KNN, 4], f32, tag="st",
                                       name="pm_ps")
                    for gi in range(4):
                        stT_v = stT[:, gi, :].rearrange("p (ql gf) -> p gf ql",
                                                        gf=KNN)
                        for g4 in range(KNN):
                            nc.tensor.matmul(pm_ps[:, gi, g4, :],
                                             mv_t[:, gi, g4, :],
                                             stT_v[:, g4, :],
                                             start=True, stop=True)
                    # copy restores query order: pm_sb cols = (ql, g4)
                    pm_sb = pms.tile([D + 1, 4, 4, KNN], f32, tag="pm",
                                     name="pm_sb")
                    nc.scalar.copy(
                        out=pm_sb,
                        in_=pm_ps.rearrange("p gi g4 ql -> p gi ql g4"))
                    # combine: transposed-accumulate the mem numerator and
                    # denominator into psum_o, then oh = psum[:64] / psum[64]
                    oh_ps = ppt.tile([D, 4, P], f16, tag="tps", name="oh_ps")
                    for gi in range(4):
                        nc.tensor.matmul(
                            psum_o[:, gi, :],
                            pm_sb[:, gi].rearrange("p ql g4 -> p (ql g4)"),
                            eye32_sb, is_transpose=True,
                            start=False, stop=(gi == 3))
                    for gi in range(4):
                        rcp = smallp.tile([P, 1], f32, tag="rcp", name="rcp")
                        nc.vector.reciprocal(rcp, psum_o[:, gi, D:D + 1])
                        oh = smallp.tile([P, D], f16, tag="oh", name="oh")
                        nc.vector.tensor_scalar_mul(oh, psum_o[:, gi, 0:D],
                                                    rcp)
                        nc.tensor.transpose(oh_ps[:, gi, :], oh, eye_sb)
                    nc.scalar.copy(
                        out=hoT[p * D:(p + 1) * D, 4 * qc:4 * qc + 4, :],
                        in_=oh_ps)
                    if p == 1:
                        for gi in range(4):
                            g = 4 * qc + gi
                            pf = pp512.tile([P, DIM], f32, tag="st",
                                            name="pf")
                            nc.tensor.matmul(pf, hoT[:, g, :], wout_sb,
                                             start=True, stop=True)
                            of_s = outp.tile([P, DIM], f16, tag="ofs",
                                             name="of_s")
                            nc.scalar.copy(out=of_s, in_=pf)
                            nc.sync.dma_start(out[g * P:(g + 1) * P, :], of_s)

    _mark(nc, "tile_finish")
    nc.compile()
    _mark(nc, None)
    return nc


def _prep_mv(mv_slice, bf16):
    """[2,2048,32,64] -> [2,16,128,32,65] bf16: partition (ql j) stacks the 4
    stride-32 queries of each group; col 64 = 1.0 (softmax-denominator row)."""
    r = mv_slice.reshape(2, NB, 4, KNN, KNN, D).transpose(0, 1, 2, 4, 3, 5)
    out = np.empty((2, NB, P, KNN, D + 1), dtype=bf16)
    out[..., :D] = r.reshape(2, NB, P, KNN, D).astype(bf16)
    out[..., D] = 1.0
    return np.ascontiguousarray(out)


def _prepare_in_maps(x, w_q, w_kv, w_out, scale_param, mem_k, mem_v, mem_mask,
                     use_mbias):
    import ml_dtypes
    f = np.float32
    f16 = np.float16
    bf16 = ml_dtypes.bfloat16
    scales8 = np.exp(scale_param.reshape(HEADS).astype(f))
    in_maps = []
    for c in range(NCORES):
        b = c // 4
        h0 = 2 * (c % 4)
        # xT: [p, co, n] = x[n, co*128 + p]
        xT = np.ascontiguousarray(
            x[b].T.reshape(NCO, P, N).transpose(1, 0, 2).astype(f16))
        wq_h = w_q[:, h0 * D:(h0 + 2) * D]
        wcat = np.concatenate([wq_h, w_kv], axis=1)            # [512, 256]
        wqkv = np.ascontiguousarray(
            wcat.reshape(NCO, P, 4 * D).astype(f16))
        wqkv = np.ascontiguousarray(wqkv.transpose(1, 0, 2))   # [p, co, 256]
        sc = np.empty((P, 8), dtype=f)
        sc[:, 0] = scales8[h0]
        sc[:, 1] = scales8[h0 + 1]
        sc[:, 2] = -scales8[h0]
        sc[:, 3] = -scales8[h0 + 1]
        sc[:, 4] = 1.0 / scales8[h0] ** 2
        sc[:, 5] = 1.0 / scales8[h0 + 1] ** 2
        sc[:, 6:8] = 0.0
        m = {
            "xt": xT,
            "wqkv": wqkv,
            "wout2": np.ascontiguousarray(
                w_out[h0 * D:(h0 + 2) * D, :].astype(f16)),
            "scvec": sc,
            "mk": np.ascontiguousarray(
                mem_k[b, h0:h0 + 2].reshape(2, NB, P, KNN, D).astype(f16)),
            "mv": _prep_mv(mem_v[b, h0:h0 + 2], bf16),
        }
        if use_mbias:
            mb = np.where(mem_mask[b, h0:h0 + 2], f(0), f(-1e30)).astype(f)
            m["mbias"] = np.ascontiguousarray(mb.reshape(2, NB, P, KNN))
        in_maps.append(m)
    return in_maps


def _run(x, w_q, w_kv, w_out, scale_param, mem_k, mem_v, mem_mask, trace=False):
    from concourse.bass_utils import run_bass_kernel_spmd

    use_mbias = not bool(np.all(mem_mask))
    nc = _build(use_mbias)
    in_maps = _prepare_in_maps(x, w_q, w_kv, w_out, scale_param,
                               mem_k, mem_v, mem_mask, use_mbias)
    res = run_bass_kernel_spmd(nc, in_maps, core_ids=list(range(NCORES)),
                               trace=trace)
    out = np.zeros((B, N, DIM), dtype=np.float32)
    for c in range(NCORES):
        out[c // 4] += res.results[c]["out"].astype(np.float32)
    return out, res


def kernel(x, w_q, w_kv, w_out, scale_param, mem_k, mem_v, mem_mask):
    trace = bool(int(os.environ.get("BASS_KERNEL_TRACE", "0")))
    out, _ = _run(x, w_q, w_kv, w_out, scale_param, mem_k, mem_v, mem_mask,
                  trace=trace)
    return out


# revision 24
# speedup vs baseline: 1.0113x; 1.0113x over previous
"""Trainium2 Bass kernel for nn_MemorizingTransformer (retrieval_knn).

Memorizing-transformer attention block: cosine-sim causal local attention with
per-query retrieved KNN memories, joint softmax over [memory | local], and
input/output projections.

Sharding: (b, h) across 8 cores — core c handles batch b=c//4 and heads
h0=2*(c%4), h0+1. Every core runs an identical NEFF (pure SPMD); only input
slices differ. The output projection is computed per-core on the core's two
head rows of w_out, giving fp16 partial sums that the host reduces.

Precision plan (validated against the reference on CPU, rel-err ~5e-3 vs the
2e-2 gate): fp16 for the score path (x, w_q/w_kv, q̂, k, mem_k, mem-score
products and their f16-accumulated reduce), bf16 for everything downstream of
exp (probabilities, v, mem_v, mem-value products), f32 for PSUM accumulation
and softmax denominators. exp outputs must be bf16, not fp16: scores span
exp(-2*scale) .. 1 which underflows fp16's exponent range.

Device algorithm per core (all 16-bit matmuls, 1 cycle/row on PE):
  proj   : qkv = xT-chunks @ wqkv (PE, f32 psum); ACT copies to SBUF f16/bf16;
           sumsq via DVE tensor_tensor_reduce; q normalized on DVE.
           k stays unnormalized — 1/|k_j| is folded into the exp scale, which
           is per-partition (per j) in the S_T = [j, q] layout.
  qT/kT  : PE transposes (fp16 identity), psum -> SBUF via gpsimd copies.
  per head p, per query-quad qc (4 blocks of 128 queries):
    memk : DMA mk f16 -> DVE mul with q̂ (2x) -> DVE f16 reduce (2x) -> ACT
           exp -> P_mem bf16.
    local: per jt <= 4qc+3: S_T[128j, <=512q] = kT_jt.T @ qT (PE), ACT exp
           with scale=sc/|k_j| bias=-sc -> p_t bf16, tril mask on the
           diagonal block (DVE), PV: psum_o[g] += p_t.T @ [v|1] (PE accum,
           stop at jt==g).
    memv : DMA mv bf16 (d-major) -> DVE mul with P_mem -> DVE reduce -> mem
           numerator; denominator joined via tensor_tensor_reduce seeded with
           the local denominator from psum col 64; oh = (psum+mo)*rcp (DVE).
    ohT  : PE transposes -> hoT f16.
  outproj: partial[g] = hoT_g.T @ w_out rows (PE), ACT copy f16, DMA out.

Softmax needs no max-subtraction: scores are cosine sims in [-1,1] times
scale=exp(scale_param), so exp(scale*(s-1)) is bounded in (0, 1].
"""

import os
import numpy as np

HEADS = 8
D = 64
KNN = 32
B = 2
N = 2048
DIM = 512
P = 128
NB = N // P          # 16 query/key blocks
NCO = DIM // P       # 4 contraction chunks of the model dim
NCORES = 8
REDV_F32 = bool(int(os.environ.get("BASS_REDV_F32", "0")))
MK_G = int(os.environ.get("BASS_MK_G", "14"))   # memk reduces on gpsimd (of 32)
MV_G = int(os.environ.get("BASS_MV_G", "14"))   # memv reduces on gpsimd (of 32)
PHASE_MARKS = []
_MSTATE = {}


def _mark(nc, name):
    cur = nc.next_id()
    if _MSTATE.get("name") is not None:
        PHASE_MARKS.append((_MSTATE["name"], _MSTATE["id"], cur))
    _MSTATE["name"] = name
    _MSTATE["id"] = cur


def _build(use_mbias: bool):
    import concourse.bass as bass
    import concourse.mybir as mybir
    import concourse.tile as tile
    from concourse import bacc

    f32 = mybir.dt.float32
    f16 = mybir.dt.float16
    bf16 = mybir.dt.bfloat16
    mvdt = f32 if REDV_F32 else bf16
    AX = mybir.AxisListType
    ACTF = mybir.ActivationFunctionType
    ALU = mybir.AluOpType

    nc = bacc.Bacc(None, target_bir_lowering=False, name="memxformer")
    PHASE_MARKS.clear()
    _MSTATE.clear()

    # ---- I/O ------------------------------------------------------------
    xt = nc.dram_tensor("xt", (P, NCO, N), f16, kind="ExternalInput")
    wqkv = nc.dram_tensor("wqkv", (P, NCO, 4 * D), f16, kind="ExternalInput")
    wout2 = nc.dram_tensor("wout2", (2 * D, DIM), f16, kind="ExternalInput")
    # scvec cols: [sc0, sc1, -sc0, -sc1, 1/sc0^2, 1/sc1^2, 0, 0]
    scvec = nc.dram_tensor("scvec", (P, 8), f32, kind="ExternalInput")
    mk = nc.dram_tensor("mk", (2, NB, P, KNN, D), f16, kind="ExternalInput")
    mv = nc.dram_tensor("mv", (2, NB, P, KNN, D + 1), bf16, kind="ExternalInput")
    if use_mbias:
        mbias = nc.dram_tensor("mbias", (2, NB, P, KNN), f32, kind="ExternalInput")
    out = nc.dram_tensor("out", (N, DIM), f16, kind="ExternalOutput")

    # constants baked into the NEFF
    import ml_dtypes
    eye16_d = nc.inline_tensor(np.eye(P, dtype=np.float16), name="eye16_c")
    eyebf_d = nc.inline_tensor(np.eye(P, dtype=ml_dtypes.bfloat16), name="eyebf_c")
    eye32_d = nc.inline_tensor(np.eye(D + 1, dtype=np.float32), name="eye32_c")
    # keep j <= q in the S_T = [j, q] layout
    triu_d = nc.inline_tensor(
        np.triu(np.ones((P, P), dtype=np.float32)).astype(ml_dtypes.bfloat16),
        name="triu_c")


    with tile.TileContext(nc) as tc:
        with (
            tc.tile_pool(name="singles", bufs=1) as singles,
            tc.tile_pool(name="mkp", bufs=2) as mkp,
            tc.tile_pool(name="mvp", bufs=2) as mvp,
            tc.tile_pool(name="prodp", bufs=2) as prodp,
            tc.tile_pool(name="treep", bufs=1) as treep,
            tc.tile_pool(name="ptp", bufs=3) as ptp,
            tc.tile_pool(name="stts", bufs=2) as stts,
            tc.tile_pool(name="pms", bufs=2) as pms,
            tc.tile_pool(name="smallp", bufs=6) as smallp,
            tc.tile_pool(name="outp", bufs=2) as outp,
            tc.tile_pool(name="ppt", bufs=2, space="PSUM") as ppt,
            tc.tile_pool(name="pp512", bufs=2, space="PSUM") as pp512,
            tc.tile_pool(name="ppo", bufs=4, space="PSUM") as ppo,
        ):
            # ---- constants / weights ------------------------------------
            eye_sb = singles.tile([P, P], f16, tag="eye16")
            nc.sync.dma_start(eye_sb, eye16_d[:, :])
            eyebf_sb = singles.tile([P, P], bf16, tag="eyebf")
            nc.sync.dma_start(eyebf_sb, eyebf_d[:, :])
            eye32_sb = singles.tile([D + 1, D + 1], f32, tag="eye32")
            nc.sync.dma_start(eye32_sb, eye32_d[:, :])
            tril_sb = singles.tile([P, P], bf16, tag="tril")
            nc.sync.dma_start(tril_sb, triu_d[:, :])
            sc_sb = singles.tile([P, 8], f32, tag="scales")
            nc.sync.dma_start(sc_sb, scvec[:, :])
            wqkv_sb = singles.tile([P, NCO, 4 * D], f16, tag="wqkv")
            nc.sync.dma_start(wqkv_sb, wqkv[:, :, :])
            wout_sb = singles.tile([P, DIM], f16, tag="wout")
            nc.sync.dma_start(wout_sb, wout2[:, :])
            xT = singles.tile([P, NCO, N], f16, tag="xT")
            nc.sync.dma_start(xT, xt[:, :, :])

            _mark(nc, "setup")
            # ---- projections: qkv = x @ [wq2 | wk | wv] -----------------
            # qkv16 cols: [q0 64 | q1 64 | k 64]; v goes straight to v_aug
            qkv16 = singles.tile([P, NB, 3 * D], f16, tag="qkv16")
            v_aug = singles.tile([P, NB, D + 1], bf16, tag="vaug")
            nc.gpsimd.memset(v_aug[:, :, D:D + 1], 1.0)
            # ss cols: 0:32 q-blocks (2 heads x 16), 32:48 k-blocks
            ss = singles.tile([P, 3 * NB], f32, tag="ss")
            junk = singles.tile([P, D], f16, tag="junk")

            for g in range(NB):
                qsl = slice(g * P, (g + 1) * P)
                qkv_ps = ppt.tile([P, 4 * D], f32, tag="tps")
                for co in range(NCO):
                    nc.tensor.matmul(qkv_ps, xT[:, co, qsl], wqkv_sb[:, co, :],
                                     start=(co == 0), stop=(co == NCO - 1))
                nc.scalar.copy(out=qkv16[:, g, :], in_=qkv_ps[:, 0:3 * D])
                nc.scalar.copy(out=v_aug[:, g, 0:D], in_=qkv_ps[:, 3 * D:4 * D])
                for s3 in range(3):
                    col = s3 * NB + g if s3 < 2 else 2 * NB + g
                    seg = qkv16[:, g, s3 * D:(s3 + 1) * D]
                    nc.scalar.activation(out=junk, in_=seg, func=ACTF.Square,
                                         accum_out=ss[:, col:col + 1])

            _mark(nc, "proj")
            # ---- norms --------------------------------------------------
            # q: q_s = q / |q|;  k: fold sc/|k_j| into the local exp scale
            nrm_q = singles.tile([P, 2 * NB], f32, tag="nrm_q")
            nc.scalar.sqrt(nrm_q, ss[:, 0:2 * NB])
            rn_q = singles.tile([P, 2 * NB], f32, tag="rn_q")
            nc.vector.reciprocal(rn_q, nrm_q)
            scexp = singles.tile([P, 2, NB], f32, tag="scexp")
            sktmp = singles.tile([P, 2, NB], f32, tag="sktmp")
            for p in range(2):
                nc.scalar.activation(out=sktmp[:, p, :],
                                     in_=ss[:, 2 * NB:3 * NB],
                                     func=ACTF.Sqrt,
                                     scale=sc_sb[:, 4 + p:5 + p])
                nc.vector.reciprocal(scexp[:, p, :], sktmp[:, p, :])

            q_s = singles.tile([P, 2 * NB, D], f16, tag="q_s")
            for idx in range(2 * NB):
                p, g = idx // NB, idx % NB
                nc.vector.tensor_scalar_mul(q_s[:, idx, :],
                                            qkv16[:, g, p * D:(p + 1) * D],
                                            rn_q[:, idx:idx + 1])

            _mark(nc, "norm")
            # ---- transposes: qT, kT -------------------------------------
            qT = singles.tile([D, 2 * NB, P], f16, tag="qT")
            for idxc in range(0, 2 * NB, 4):
                qt_ps = ppt.tile([D, 4, P], f16, tag="tps")
                for i4 in range(4):
                    nc.tensor.transpose(qt_ps[:, i4, :], q_s[:, idxc + i4, :],
                                        eye_sb)
                nc.scalar.copy(out=qT[:, idxc:idxc + 4, :], in_=qt_ps)
            kT = singles.tile([D, NB, P], f16, tag="kT")
            for jtc in range(0, NB, 4):
                kt_ps = ppt.tile([D, 4, P], f16, tag="tps")
                for j4 in range(4):
                    nc.tensor.transpose(kt_ps[:, j4, :],
                                        qkv16[:, jtc + j4, 2 * D:3 * D], eye_sb)
                nc.scalar.copy(out=kT[:, jtc:jtc + 4, :], in_=kt_ps)

            _mark(nc, "qkT")
            # ---- attention ----------------------------------------------
            hoT = singles.tile([P, NB, P], f16, tag="hoT")
            pm_all = singles.tile([P, 2, NB, KNN], bf16, tag="pm_all")
            # block-diagonal staging for P_mem (the off-diagonal zeros are
            # written once and never touched again; double-buffered per quad)
            st2 = singles.tile([P, 2, 4, P], bf16, tag="st2")
            nc.gpsimd.memset(st2, 0.0)

            for p in range(2):
                sc_ap = sc_sb[:, p:p + 1]
                nb_ap = sc_sb[:, 2 + p:3 + p]

                def memk_quad(qc):
                    gc = 4 * qc
                    mk_t = mkp.tile([P, 4, KNN, D], f16, tag="mk", name="mk_t")
                    nc.sync.dma_start(
                        mk_t, mk[p, gc:gc + 4].rearrange("g p j d -> p g j d"))
                    prod = prodp.tile([P, 4, KNN, D], f16, tag="prod",
                                      name="prod")
                    nc.vector.tensor_mul(
                        prod, mk_t,
                        q_s[:, p * NB + gc:p * NB + gc + 4, None, :]
                        .to_broadcast((P, 4, KNN, D)))
                    t1 = treep.tile([P, 4, KNN, D // 2], f16, tag="t1",
                                    name="t1")
                    nc.vector.tensor_add(t1, prod[:, :, :, 0:D // 2],
                                         prod[:, :, :, D // 2:D])
                    t2 = treep.tile([P, 4, KNN, D // 4], f16, tag="t2",
                                    name="t2")
                    nc.vector.tensor_add(t2, t1[:, :, :, 0:D // 4],
                                         t1[:, :, :, D // 4:D // 2])
                    s_mem = smallp.tile([P, 4, KNN], f32, tag="smem",
                                        name="s_mem")
                    nc.vector.reduce_sum(s_mem, t2, axis=AX.X)
                    if use_mbias:
                        mb_t = smallp.tile([P, 4, KNN], f32, tag="mbias",
                                           name="mb_t")
                        nc.sync.dma_start(
                            mb_t,
                            mbias[p, gc:gc + 4].rearrange("g p j -> p g j"))
                        nc.vector.tensor_add(s_mem, s_mem, mb_t)
                    nc.scalar.activation(out=pm_all[:, p, gc:gc + 4, :],
                                         in_=s_mem, func=ACTF.Exp,
                                         bias=nb_ap, scale=sc_ap)

                def local_tile(qc, jt, psum_o):
                    g_lo = max(jt, 4 * qc)
                    g_hi = 4 * qc + 4
                    ng = g_hi - g_lo
                    i_lo = p * NB + g_lo
                    st_ps = pp512.tile([P, 512], f32, tag="st", name="st_ps")
                    nc.tensor.matmul(
                        st_ps[:, :ng * P], kT[:, jt, :],
                        qT[:, i_lo:i_lo + ng, :], start=True, stop=True)
                    p_t = ptp.tile([P, 4, P], bf16, tag="pt", name="p_t")
                    nc.scalar.activation(
                        out=p_t[:, :ng, :],
                        in_=st_ps[:, :ng * P].rearrange("p (g q) -> p g q", q=P),
                        func=ACTF.Exp, bias=nb_ap, scale=scexp[:, p, jt:jt + 1])
                    if g_lo <= jt < g_hi:
                        di = jt - g_lo
                        nc.vector.tensor_mul(p_t[:, di, :], p_t[:, di, :],
                                             tril_sb)
                    for gi in range(ng):
                        g = g_lo + gi
                        nc.tensor.matmul(
                            psum_o[:, g - 4 * qc, :], p_t[:, gi, :],
                            v_aug[:, jt, :],
                            start=(jt == 0 and gi == 0), stop=False)

                _mark(nc, "attn")
                for qc in range(4):
                    memk_quad(qc)
                    gc = 4 * qc
                    # mv prefetch + P_mem staging run on DMA/gpsimd/PE ahead
                    # of (and overlapped with) the local S/PV chain
                    mv_t = mvp.tile([P, 4, KNN, D + 1], bf16, tag="mv",
                                    name="mv_t")
                    nc.sync.dma_start(
                        mv_t, mv[p, gc:gc + 4].rearrange("g p q d -> p g q d"))
                    stage4 = st2[:, qc % 2, :, :]
                    for gi in range(4):
                        g = gc + gi
                        for k4 in range(4):
                            nc.gpsimd.tensor_copy(
                                out=stage4[32 * k4:32 * (k4 + 1), gi,
                                           32 * k4:32 * (k4 + 1)],
                                in_=pm_all[32 * k4:32 * (k4 + 1), p, g, :])
                    stt_ps = ppt.tile([P, 4, P], bf16, tag="tps", name="stt_ps")
                    for gi in range(4):
                        nc.tensor.transpose(stt_ps[:, gi, :], stage4[:, gi, :],
                                            eyebf_sb)
                    stT = stts.tile([P, 4, P], bf16, tag="stT", name="stT")
                    nc.scalar.copy(out=stT, in_=stt_ps)
                    psum_o = ppo.tile([P, 4, D + 1], f32, tag="po",
                                      name=f"po{qc}")
                    for jt in range(4 * qc + 4):
                        local_tile(qc, jt, psum_o)
                    # pm_ps columns are (g4, ql) so each tiny matmul writes
                    # 4 contiguous columns; the accumulate-transpose below
                    # restores query order ql*32+g4 via a rearranged AP
                    pm_ps = pp512.tile([D + 1, 4, KNN, 4], f32, tag="st",
                                       name="pm_ps")
                    for gi in range(4):
                        stT_v = stT[:, gi, :].rearrange("p (ql gf) -> p gf ql",
                                                        gf=KNN)
                        for g4 in range(KNN):
                            nc.tensor.matmul(pm_ps[:, gi, g4, :],
                                             mv_t[:, gi, g4, :],
                                             stT_v[:, g4, :],
                                             start=True, stop=True)
                    # copy restores query order: pm_sb cols = (ql, g4)
                    pm_sb = pms.tile([D + 1, 4, 4, KNN], f32, tag="pm",
                                     name="pm_sb")
                    nc.scalar.copy(
                        out=pm_sb,
                        in_=pm_ps.rearrange("p gi g4 ql -> p gi ql g4"))
                    # combine: transposed-accumulate the mem numerator and
                    # denominator into psum_o, then oh = psum[:64] / psum[64]
                    oh_ps = ppt.tile([D, 4, P], f16, tag="tps", name="oh_ps")
                    for gi in range(4):
                        nc.tensor.matmul(
                            psum_o[:, gi, :],
                            pm_sb[:, gi].rearrange("p ql g4 -> p (ql g4)"),
                            eye32_sb, is_transpose=True,
                            start=False, stop=(gi == 3))
                    for gi in range(4):
                        rcp = smallp.tile([P, 1], f32, tag="rcp", name="rcp")
                        nc.vector.reciprocal(rcp, psum_o[:, gi, D:D + 1])
                        oh = smallp.tile([P, D], f16, tag="oh", name="oh")
                        nc.vector.tensor_scalar_mul(oh, psum_o[:, gi, 0:D],
                                                    rcp)
                        nc.tensor.transpose(oh_ps[:, gi, :], oh, eye_sb)
                    nc.scalar.copy(
                        out=hoT[p * D:(p + 1) * D, 4 * qc:4 * qc + 4, :],
                        in_=oh_ps)
                    if p == 1:
                        for gi in range(4):
                            g = 4 * qc + gi
                            pf = pp512.tile([P, DIM], f32, tag="st",
                                            name="pf")
                            nc.tensor.matmul(pf, hoT[:, g, :], wout_sb,
                                             start=True, stop=True)
                            of_s = outp.tile([P, DIM], f16, tag="ofs",
                                             name="of_s")
                            nc.scalar.copy(out=of_s, in_=pf)
                            nc.sync.dma_start(out[g * P:(g + 1) * P, :], of_s)

    _mark(nc, "tile_finish")
    nc.compile()
    _mark(nc, None)
    return nc


def _prep_mv(mv_slice, bf16):
    """[2,2048,32,64] -> [2,16,128,32,65] bf16: partition (ql j) stacks the 4
    stride-32 queries of each group; col 64 = 1.0 (softmax-denominator row)."""
    r = mv_slice.reshape(2, NB, 4, KNN, KNN, D).transpose(0, 1, 2, 4, 3, 5)
    out = np.empty((2, NB, P, KNN, D + 1), dtype=bf16)
    out[..., :D] = r.reshape(2, NB, P, KNN, D).astype(bf16)
    out[..., D] = 1.0
    return np.ascontiguousarray(out)


def _prepare_in_maps(x, w_q, w_kv, w_out, scale_param, mem_k, mem_v, mem_mask,
                     use_mbias):
    import ml_dtypes
    f = np.float32
    f16 = np.float16
    bf16 = ml_dtypes.bfloat16
    scales8 = np.exp(scale_param.reshape(HEADS).astype(f))
    in_maps = []
    for c in range(NCORES):
        b = c // 4
        h0 = 2 * (c % 4)
        # xT: [p, co, n] = x[n, co*128 + p]
        xT = np.ascontiguousarray(
            x[b].T.reshape(NCO, P, N).transpose(1, 0, 2).astype(f16))
        wq_h = w_q[:, h0 * D:(h0 + 2) * D]
        wcat = np.concatenate([wq_h, w_kv], axis=1)            # [512, 256]
        wqkv = np.ascontiguousarray(
            wcat.reshape(NCO, P, 4 * D).astype(f16))
        wqkv = np.ascontiguousarray(wqkv.transpose(1, 0, 2))   # [p, co, 256]
        sc = np.empty((P, 8), dtype=f)
        sc[:, 0] = scales8[h0]
        sc[:, 1] = scales8[h0 + 1]
        sc[:, 2] = -scales8[h0]
        sc[:, 3] = -scales8[h0 + 1]
        sc[:, 4] = 1.0 / scales8[h0] ** 2
        sc[:, 5] = 1.0 / scales8[h0 + 1] ** 2
        sc[:, 6:8] = 0.0
        m = {
            "xt": xT,
            "wqkv": wqkv,
            "wout2": np.ascontiguousarray(
                w_out[h0 * D:(h0 + 2) * D, :].astype(f16)),
            "scvec": sc,
            "mk": np.ascontiguousarray(
                mem_k[b, h0:h0 + 2].reshape(2, NB, P, KNN, D).astype(f16)),
            "mv": _prep_mv(mem_v[b, h0:h0 + 2], bf16),
        }
        if use_mbias:
            mb = np.where(mem_mask[b, h0:h0 + 2], f(0), f(-1e30)).astype(f)
            m["mbias"] = np.ascontiguousarray(mb.reshape(2, NB, P, KNN))
        in_maps.append(m)
    return in_maps


def _run(x, w_q, w_kv, w_out, scale_param, mem_k, mem_v, mem_mask, trace=False):
    from concourse.bass_utils import run_bass_kernel_spmd

    use_mbias = not bool(np.all(mem_mask))
    nc = _build(use_mbias)
    in_maps = _prepare_in_maps(x, w_q, w_kv, w_out, scale_param,
                               mem_k, mem_v, mem_mask, use_mbias)
    res = run_bass_kernel_spmd(nc, in_maps, core_ids=list(range(NCORES)),
                               trace=trace)
    out = np.zeros((B, N, DIM), dtype=np.float32)
    for c in range(NCORES):
        out[c // 4] += res.results[c]["out"].astype(np.float32)
    return out, res


def kernel(x, w_q, w_kv, w_out, scale_param, mem_k, mem_v, mem_mask):
    trace = bool(int(os.environ.get("BASS_KERNEL_TRACE", "0")))
    out, _ = _run(x, w_q, w_kv, w_out, scale_param, mem_k, mem_v, mem_mask,
                  trace=trace)
    return out


# revision 25
# speedup vs baseline: 1.6840x; 1.6652x over previous
"""Trainium2 Bass kernel for nn_MemorizingTransformer (retrieval_knn).

Memorizing-transformer attention block: cosine-sim causal local attention with
per-query retrieved KNN memories, joint softmax over [memory | local], and
input/output projections.

Sharding: (b, h) across 8 cores — core c handles batch b=c//4 and heads
h0=2*(c%4), h0+1. Every core runs an identical NEFF (pure SPMD); only input
slices differ. The output projection is computed per-core on the core's two
head rows of w_out, giving fp16 partial sums that the host reduces.

Precision plan (validated against the reference on CPU, rel-err ~5e-3 vs the
2e-2 gate): fp16 for the score path (x, w_q/w_kv, q̂, k, mem_k, mem-score
products and their f16-accumulated reduce), bf16 for everything downstream of
exp (probabilities, v, mem_v, mem-value products), f32 for PSUM accumulation
and softmax denominators. exp outputs must be bf16, not fp16: scores span
exp(-2*scale) .. 1 which underflows fp16's exponent range.

Device algorithm per core (all 16-bit matmuls, 1 cycle/row on PE):
  proj   : qkv = xT-chunks @ wqkv (PE, f32 psum); ACT copies to SBUF f16/bf16;
           sumsq via ACT Square+accum; q normalized on DVE. k stays
           unnormalized - 1/|k_j| is folded into the exp scale, which is
           per-partition (per j) in the S_T = [j, q] layout.
  qT/kT  : PE transposes (fp16 identity), psum -> SBUF via ACT copies.
  per head p, per query-quad qc (4 blocks of 128 queries):
    memk : DMA mk f16 (quad-batched) -> one DVE mul with q-hat (2x mode) ->
           two f16 tree-add levels (2x) -> short f32 reduce -> ACT exp ->
           P_mem bf16.
    stage: P_mem staged block-diagonally (gpsimd copies), transposed on PE.
    local: per jt <= 4qc+3: S_T[128j, <=512q] = kT_jt.T @ qT (PE), ACT exp
           with scale=sc/|k_j| bias=-sc -> p_t bf16, tril mask on the
           diagonal block (DVE), PV: psum_o[g] += p_t.T @ [v|1] (PE accum).
    memv : PE block-diagonal trick: 32 tiny matmuls per block give
           pm[65, q] = [mem_v|1].T @ P_mem per query group ((g4, ql)-ordered
           so each matmul writes contiguous psum columns); ACT copy restores
           query order; transposed-accumulate into psum_o so col 64 = total
           softmax denominator.
    comb : oh = psum_o[:, :64] * recip(psum_o[:, 64]) (DVE); hoT via PE
           transpose.
  outproj: partial[g] = hoT_g.T @ w_out rows (PE), ACT copy f16, DMA out.

Softmax needs no max-subtraction: scores are cosine sims in [-1,1] times
scale=exp(scale_param), so exp(scale*(s-1)) is bounded in (0, 1].
"""

import os
import numpy as np

HEADS = 8
D = 64
KNN = 32
B = 2
N = 2048
DIM = 512
P = 128
NB = N // P          # 16 query/key blocks
NCO = DIM // P       # 4 contraction chunks of the model dim
NCORES = 8
REDV_F32 = bool(int(os.environ.get("BASS_REDV_F32", "0")))
MK_G = int(os.environ.get("BASS_MK_G", "14"))   # memk reduces on gpsimd (of 32)
MV_G = int(os.environ.get("BASS_MV_G", "14"))   # memv reduces on gpsimd (of 32)
PHASE_MARKS = []
_MSTATE = {}


def _mark(nc, name):
    cur = nc.next_id()
    if _MSTATE.get("name") is not None:
        PHASE_MARKS.append((_MSTATE["name"], _MSTATE["id"], cur))
    _MSTATE["name"] = name
    _MSTATE["id"] = cur


def _build(use_mbias: bool):
    import concourse.bass as bass
    import concourse.mybir as mybir
    import concourse.tile as tile
    from concourse import bacc

    f32 = mybir.dt.float32
    f16 = mybir.dt.float16
    bf16 = mybir.dt.bfloat16
    mvdt = f32 if REDV_F32 else bf16
    AX = mybir.AxisListType
    ACTF = mybir.ActivationFunctionType
    ALU = mybir.AluOpType

    nc = bacc.Bacc(None, target_bir_lowering=False, name="memxformer")
    PHASE_MARKS.clear()
    _MSTATE.clear()

    # ---- I/O ------------------------------------------------------------
    xt = nc.dram_tensor("xt", (P, NCO, N), f16, kind="ExternalInput")
    wqkv = nc.dram_tensor("wqkv", (P, NCO, 4 * D), f16, kind="ExternalInput")
    wout2 = nc.dram_tensor("wout2", (2 * D, DIM), f16, kind="ExternalInput")
    # scvec cols: [sc0, sc1, -sc0, -sc1, 1/sc0^2, 1/sc1^2, 0, 0]
    scvec = nc.dram_tensor("scvec", (P, 8), f32, kind="ExternalInput")
    mk = nc.dram_tensor("mk", (2, NB, P, KNN, D), f16, kind="ExternalInput")
    mv = nc.dram_tensor("mv", (2, NB, P, KNN, D + 1), bf16, kind="ExternalInput")
    if use_mbias:
        mbias = nc.dram_tensor("mbias", (2, NB, P, KNN), f32, kind="ExternalInput")
    out = nc.dram_tensor("out", (N, DIM), f16, kind="ExternalOutput")

    # constants baked into the NEFF
    import ml_dtypes
    eye16_d = nc.inline_tensor(np.eye(P, dtype=np.float16), name="eye16_c")
    eyebf_d = nc.inline_tensor(np.eye(P, dtype=ml_dtypes.bfloat16), name="eyebf_c")
    eye32_d = nc.inline_tensor(np.eye(D + 1, dtype=np.float32), name="eye32_c")
    # keep j <= q in the S_T = [j, q] layout
    triu_d = nc.inline_tensor(
        np.triu(np.ones((P, P), dtype=np.float32)).astype(ml_dtypes.bfloat16),
        name="triu_c")


    with tile.TileContext(nc) as tc:
        with (
            tc.tile_pool(name="singles", bufs=1) as singles,
            tc.tile_pool(name="mkp", bufs=2) as mkp,
            tc.tile_pool(name="mvp", bufs=2) as mvp,
            tc.tile_pool(name="prodp", bufs=2) as prodp,
            tc.tile_pool(name="treep", bufs=1) as treep,
            tc.tile_pool(name="ptp", bufs=3) as ptp,
            tc.tile_pool(name="stts", bufs=2) as stts,
            tc.tile_pool(name="pms", bufs=2) as pms,
            tc.tile_pool(name="smallp", bufs=6) as smallp,
            tc.tile_pool(name="outp", bufs=2) as outp,
            tc.tile_pool(name="ppt", bufs=2, space="PSUM") as ppt,
            tc.tile_pool(name="pp512", bufs=2, space="PSUM") as pp512,
            tc.tile_pool(name="ppo", bufs=4, space="PSUM") as ppo,
        ):
            # ---- constants / weights ------------------------------------
            eye_sb = singles.tile([P, P], f16, tag="eye16")
            nc.sync.dma_start(eye_sb, eye16_d[:, :])
            eyebf_sb = singles.tile([P, P], bf16, tag="eyebf")
            nc.sync.dma_start(eyebf_sb, eyebf_d[:, :])
            eye32_sb = singles.tile([D + 1, D + 1], f32, tag="eye32")
            nc.sync.dma_start(eye32_sb, eye32_d[:, :])
            tril_sb = singles.tile([P, P], bf16, tag="tril")
            nc.sync.dma_start(tril_sb, triu_d[:, :])
            sc_sb = singles.tile([P, 8], f32, tag="scales")
            nc.sync.dma_start(sc_sb, scvec[:, :])
            wqkv_sb = singles.tile([P, NCO, 4 * D], f16, tag="wqkv")
            nc.sync.dma_start(wqkv_sb, wqkv[:, :, :])
            wout_sb = singles.tile([P, DIM], f16, tag="wout")
            nc.sync.dma_start(wout_sb, wout2[:, :])
            xT = singles.tile([P, NCO, N], f16, tag="xT")
            nc.sync.dma_start(xT, xt[:, :, :])

            _mark(nc, "setup")
            # ---- projections: qkv = x @ [wq2 | wk | wv] -----------------
            # qkv16 cols: [q0 64 | q1 64 | k 64]; v goes straight to v_aug
            qkv16 = singles.tile([P, NB, 3 * D], f16, tag="qkv16")
            v_aug = singles.tile([P, NB, D + 1], bf16, tag="vaug")
            nc.gpsimd.memset(v_aug[:, :, D:D + 1], 1.0)
            # ss cols: 0:32 q-blocks (2 heads x 16), 32:48 k-blocks
            ss = singles.tile([P, 3 * NB], f32, tag="ss")
            junk = singles.tile([P, D], f16, tag="junk")

            for g in range(NB):
                qsl = slice(g * P, (g + 1) * P)
                qkv_ps = ppt.tile([P, 4 * D], f32, tag="tps")
                for co in range(NCO):
                    nc.tensor.matmul(qkv_ps, xT[:, co, qsl], wqkv_sb[:, co, :],
                                     start=(co == 0), stop=(co == NCO - 1))
                nc.scalar.copy(out=qkv16[:, g, :], in_=qkv_ps[:, 0:3 * D])
                nc.scalar.copy(out=v_aug[:, g, 0:D], in_=qkv_ps[:, 3 * D:4 * D])
                for s3 in range(3):
                    col = s3 * NB + g if s3 < 2 else 2 * NB + g
                    seg = qkv16[:, g, s3 * D:(s3 + 1) * D]
                    nc.scalar.activation(out=junk, in_=seg, func=ACTF.Square,
                                         accum_out=ss[:, col:col + 1])

            _mark(nc, "proj")
            # ---- norms --------------------------------------------------
            # q: q_s = q / |q|;  k: fold sc/|k_j| into the local exp scale
            nrm_q = singles.tile([P, 2 * NB], f32, tag="nrm_q")
            nc.scalar.sqrt(nrm_q, ss[:, 0:2 * NB])
            rn_q = singles.tile([P, 2 * NB], f32, tag="rn_q")
            nc.vector.reciprocal(rn_q, nrm_q)
            scexp = singles.tile([P, 2, NB], f32, tag="scexp")
            sktmp = singles.tile([P, 2, NB], f32, tag="sktmp")
            for p in range(2):
                nc.scalar.activation(out=sktmp[:, p, :],
                                     in_=ss[:, 2 * NB:3 * NB],
                                     func=ACTF.Sqrt,
                                     scale=sc_sb[:, 4 + p:5 + p])
                nc.vector.reciprocal(scexp[:, p, :], sktmp[:, p, :])

            q_s = singles.tile([P, 2 * NB, D], f16, tag="q_s")
            for idx in range(2 * NB):
                p, g = idx // NB, idx % NB
                nc.vector.tensor_scalar_mul(q_s[:, idx, :],
                                            qkv16[:, g, p * D:(p + 1) * D],
                                            rn_q[:, idx:idx + 1])

            _mark(nc, "norm")
            # ---- transposes: qT, kT -------------------------------------
            qT = singles.tile([D, 2 * NB, P], f16, tag="qT")
            for idxc in range(0, 2 * NB, 4):
                qt_ps = ppt.tile([D, 4, P], f16, tag="tps")
                for i4 in range(4):
                    nc.tensor.transpose(qt_ps[:, i4, :], q_s[:, idxc + i4, :],
                                        eye_sb)
                nc.scalar.copy(out=qT[:, idxc:idxc + 4, :], in_=qt_ps)
            kT = singles.tile([D, NB, P], f16, tag="kT")
            for jtc in range(0, NB, 4):
                kt_ps = ppt.tile([D, 4, P], f16, tag="tps")
                for j4 in range(4):
                    nc.tensor.transpose(kt_ps[:, j4, :],
                                        qkv16[:, jtc + j4, 2 * D:3 * D], eye_sb)
                nc.scalar.copy(out=kT[:, jtc:jtc + 4, :], in_=kt_ps)

            _mark(nc, "qkT")
            # ---- attention ----------------------------------------------
            hoT = singles.tile([P, NB, P], f16, tag="hoT")
            pm_all = singles.tile([P, 2, NB, KNN], bf16, tag="pm_all")
            # block-diagonal staging for P_mem (the off-diagonal zeros are
            # written once and never touched again; double-buffered per quad)
            st2 = singles.tile([P, 2, 4, P], bf16, tag="st2")
            nc.gpsimd.memset(st2, 0.0)

            for p in range(2):
                sc_ap = sc_sb[:, p:p + 1]
                nb_ap = sc_sb[:, 2 + p:3 + p]

                def memk_quad(qc):
                    gc = 4 * qc
                    mk_t = mkp.tile([P, 4, KNN, D], f16, tag="mk", name="mk_t")
                    nc.sync.dma_start(
                        mk_t, mk[p, gc:gc + 4].rearrange("g p j d -> p g j d"))
                    prod = prodp.tile([P, 4, KNN, D], f16, tag="prod",
                                      name="prod")
                    nc.vector.tensor_mul(
                        prod, mk_t,
                        q_s[:, p * NB + gc:p * NB + gc + 4, None, :]
                        .to_broadcast((P, 4, KNN, D)))
                    t1 = treep.tile([P, 4, KNN, D // 2], f16, tag="t1",
                                    name="t1")
                    nc.vector.tensor_add(t1, prod[:, :, :, 0:D // 2],
                                         prod[:, :, :, D // 2:D])
                    t2 = treep.tile([P, 4, KNN, D // 4], f16, tag="t2",
                                    name="t2")
                    nc.vector.tensor_add(t2, t1[:, :, :, 0:D // 4],
                                         t1[:, :, :, D // 4:D // 2])
                    s_mem = smallp.tile([P, 4, KNN], f32, tag="smem",
                                        name="s_mem")
                    nc.vector.reduce_sum(s_mem, t2, axis=AX.X)
                    if use_mbias:
                        mb_t = smallp.tile([P, 4, KNN], f32, tag="mbias",
                                           name="mb_t")
                        nc.sync.dma_start(
                            mb_t,
                            mbias[p, gc:gc + 4].rearrange("g p j -> p g j"))
                        nc.vector.tensor_add(s_mem, s_mem, mb_t)
                    nc.scalar.activation(out=pm_all[:, p, gc:gc + 4, :],
                                         in_=s_mem, func=ACTF.Exp,
                                         bias=nb_ap, scale=sc_ap)

                def local_tile(qc, jt, psum_o):
                    g_lo = max(jt, 4 * qc)
                    g_hi = 4 * qc + 4
                    ng = g_hi - g_lo
                    i_lo = p * NB + g_lo
                    st_ps = pp512.tile([P, 512], f32, tag="st", name="st_ps")
                    nc.tensor.matmul(
                        st_ps[:, :ng * P], kT[:, jt, :],
                        qT[:, i_lo:i_lo + ng, :], start=True, stop=True)
                    p_t = ptp.tile([P, 4, P], bf16, tag="pt", name="p_t")
                    nc.scalar.activation(
                        out=p_t[:, :ng, :],
                        in_=st_ps[:, :ng * P].rearrange("p (g q) -> p g q", q=P),
                        func=ACTF.Exp, bias=nb_ap, scale=scexp[:, p, jt:jt + 1])
                    if g_lo <= jt < g_hi:
                        di = jt - g_lo
                        nc.vector.tensor_mul(p_t[:, di, :], p_t[:, di, :],
                                             tril_sb)
                    for gi in range(ng):
                        g = g_lo + gi
                        nc.tensor.matmul(
                            psum_o[:, g - 4 * qc, :], p_t[:, gi, :],
                            v_aug[:, jt, :],
                            start=(jt == 0 and gi == 0), stop=False)

                _mark(nc, "attn")
                for qc in range(4):
                    memk_quad(qc)
                    gc = 4 * qc
                    # mv prefetch + P_mem staging run on DMA/gpsimd/PE ahead
                    # of (and overlapped with) the local S/PV chain
                    mv_t = mvp.tile([P, 4, KNN, D + 1], bf16, tag="mv",
                                    name="mv_t")
                    nc.sync.dma_start(
                        mv_t, mv[p, gc:gc + 4].rearrange("g p q d -> p g q d"))
                    stage4 = st2[:, qc % 2, :, :]
                    for gi in range(4):
                        g = gc + gi
                        for k4 in range(4):
                            nc.gpsimd.tensor_copy(
                                out=stage4[32 * k4:32 * (k4 + 1), gi,
                                           32 * k4:32 * (k4 + 1)],
                                in_=pm_all[32 * k4:32 * (k4 + 1), p, g, :])
                    stt_ps = ppt.tile([P, 4, P], bf16, tag="tps", name="stt_ps")
                    for gi in range(4):
                        nc.tensor.transpose(stt_ps[:, gi, :], stage4[:, gi, :],
                                            eyebf_sb)
                    stT = stts.tile([P, 4, P], bf16, tag="stT", name="stT")
                    nc.scalar.copy(out=stT, in_=stt_ps)
                    psum_o = ppo.tile([P, 4, D + 1], f32, tag="po",
                                      name=f"po{qc}")
                    for jt in range(4 * qc + 4):
                        local_tile(qc, jt, psum_o)
                    # pm_ps columns are (g4, ql) so each tiny matmul writes
                    # 4 contiguous columns; the accumulate-transpose below
                    # restores query order ql*32+g4 via a rearranged AP
                    pm_ps = pp512.tile([D + 1, 4, KNN, 4], f32, tag="st",
                                       name="pm_ps")
                    for gi in range(4):
                        stT_v = stT[:, gi, :].rearrange("p (ql gf) -> p gf ql",
                                                        gf=KNN)
                        for g4 in range(KNN):
                            nc.tensor.matmul(pm_ps[:, gi, g4, :],
                                             mv_t[:, gi, g4, :],
                                             stT_v[:, g4, :],
                                             start=True, stop=True)
                    # copy restores query order: pm_sb cols = (ql, g4)
                    pm_sb = pms.tile([D + 1, 4, 4, KNN], f32, tag="pm",
                                     name="pm_sb")
                    nc.scalar.copy(
                        out=pm_sb,
                        in_=pm_ps.rearrange("p gi g4 ql -> p gi ql g4"))
                    # combine: transposed-accumulate the mem numerator and
                    # denominator into psum_o, then oh = psum[:64] / psum[64]
                    oh_ps = ppt.tile([D, 4, P], f16, tag="tps", name="oh_ps")
                    for gi in range(4):
                        nc.tensor.matmul(
                            psum_o[:, gi, :],
                            pm_sb[:, gi].rearrange("p ql g4 -> p (ql g4)"),
                            eye32_sb, is_transpose=True,
                            start=False, stop=(gi == 3))
                    for gi in range(4):
                        rcp = smallp.tile([P, 1], f32, tag="rcp", name="rcp")
                        nc.vector.reciprocal(rcp, psum_o[:, gi, D:D + 1])
                        oh = smallp.tile([P, D], f16, tag="oh", name="oh")
                        nc.vector.tensor_scalar_mul(oh, psum_o[:, gi, 0:D],
                                                    rcp)
                        nc.tensor.transpose(oh_ps[:, gi, :], oh, eye_sb)
                    nc.scalar.copy(
                        out=hoT[p * D:(p + 1) * D, 4 * qc:4 * qc + 4, :],
                        in_=oh_ps)
                    if p == 1:
                        for gi in range(4):
                            g = 4 * qc + gi
                            pf = pp512.tile([P, DIM], f32, tag="st",
                                            name="pf")
                            nc.tensor.matmul(pf, hoT[:, g, :], wout_sb,
                                             start=True, stop=True)
                            of_s = outp.tile([P, DIM], f16, tag="ofs",
                                             name="of_s")
                            nc.scalar.copy(out=of_s, in_=pf)
                            nc.sync.dma_start(out[g * P:(g + 1) * P, :], of_s)

    _mark(nc, "tile_finish")
    nc.compile()
    _mark(nc, None)
    return nc


def _prep_mv(mv_slice, bf16):
    """[2,2048,32,64] -> [2,16,128,32,65] bf16: partition (ql j) stacks the 4
    stride-32 queries of each group; col 64 = 1.0 (softmax-denominator row)."""
    r = mv_slice.reshape(2, NB, 4, KNN, KNN, D).transpose(0, 1, 2, 4, 3, 5)
    out = np.empty((2, NB, P, KNN, D + 1), dtype=bf16)
    out[..., :D] = r.reshape(2, NB, P, KNN, D).astype(bf16)
    out[..., D] = 1.0
    return np.ascontiguousarray(out)


def _prepare_in_maps(x, w_q, w_kv, w_out, scale_param, mem_k, mem_v, mem_mask,
                     use_mbias):
    import ml_dtypes
    f = np.float32
    f16 = np.float16
    bf16 = ml_dtypes.bfloat16
    scales8 = np.exp(scale_param.reshape(HEADS).astype(f))
    in_maps = []
    for c in range(NCORES):
        b = c // 4
        h0 = 2 * (c % 4)
        # xT: [p, co, n] = x[n, co*128 + p]
        xT = np.ascontiguousarray(
            x[b].T.reshape(NCO, P, N).transpose(1, 0, 2).astype(f16))
        wq_h = w_q[:, h0 * D:(h0 + 2) * D]
        wcat = np.concatenate([wq_h, w_kv], axis=1)            # [512, 256]
        wqkv = np.ascontiguousarray(
            wcat.reshape(NCO, P, 4 * D).astype(f16))
        wqkv = np.ascontiguousarray(wqkv.transpose(1, 0, 2))   # [p, co, 256]
        sc = np.empty((P, 8), dtype=f)
        sc[:, 0] = scales8[h0]
        sc[:, 1] = scales8[h0 + 1]
        sc[:, 2] = -scales8[h0]
        sc[:, 3] = -scales8[h0 + 1]
        sc[:, 4] = 1.0 / scales8[h0] ** 2
        sc[:, 5] = 1.0 / scales8[h0 + 1] ** 2
        sc[:, 6:8] = 0.0
        m = {
            "xt": xT,
            "wqkv": wqkv,
            "wout2": np.ascontiguousarray(
                w_out[h0 * D:(h0 + 2) * D, :].astype(f16)),
            "scvec": sc,
            "mk": np.ascontiguousarray(
                mem_k[b, h0:h0 + 2].reshape(2, NB, P, KNN, D).astype(f16)),
            "mv": _prep_mv(mem_v[b, h0:h0 + 2], bf16),
        }
        if use_mbias:
            mb = np.where(mem_mask[b, h0:h0 + 2], f(0), f(-1e30)).astype(f)
            m["mbias"] = np.ascontiguousarray(mb.reshape(2, NB, P, KNN))
        in_maps.append(m)
    return in_maps


def _run(x, w_q, w_kv, w_out, scale_param, mem_k, mem_v, mem_mask, trace=False):
    from concourse.bass_utils import run_bass_kernel_spmd

    use_mbias = not bool(np.all(mem_mask))
    nc = _build(use_mbias)
    in_maps = _prepare_in_maps(x, w_q, w_kv, w_out, scale_param,
                               mem_k, mem_v, mem_mask, use_mbias)
    res = run_bass_kernel_spmd(nc, in_maps, core_ids=list(range(NCORES)),
                               trace=trace)
    out = np.zeros((B, N, DIM), dtype=np.float32)
    for c in range(NCORES):
        out[c // 4] += res.results[c]["out"].astype(np.float32)
    return out, res


def kernel(x, w_q, w_kv, w_out, scale_param, mem_k, mem_v, mem_mask):
    trace = bool(int(os.environ.get("BASS_KERNEL_TRACE", "0")))
    out, _ = _run(x, w_q, w_kv, w_out, scale_param, mem_k, mem_v, mem_mask,
                  trace=trace)
    return out
